# revision 27
# baseline (speedup 1.0000x reference)
"""BiLSTM (B=64, L=256, D=512, H=512) on 8 Trainium2 NeuronCores.

Strategy: 8 cores = 2 directions x 4 batch-slices of 16 (weights replicated
per direction, sequential time loop local to each core — no cross-core
communication).  Backward-direction cores receive time-reversed x, so every
core runs the identical SPMD program.

v7 (surgical over the v2 baseline):
  - bf16 matmuls, fp32 PSUM; g-gate weights pre-scaled 2x so ONE sigmoid
    covers all four gate strips (tanh(x) = 2*sig(2x)-1).
  - single full-width tanh / h-mul / out-DMA per step (baseline split them
    into halves);
  - all 4 h-transposes land in ONE psum tile, moved by ONE DVE copy;
  - xpart-slice staging and out_h DMAs issued from GpSimd, phase-1 x
    tile loads as one rearranged DMA from GpSimd (keeps SP/ACT free);
  - phase-1 GEMM interleaved with the recurrence for PE warmth.
"""

import numpy as np
import ml_dtypes

from concourse import tile, mybir, bacc
from concourse.bass_utils import run_bass_kernel_spmd
from concourse.masks import make_identity

FP = mybir.dt.float32
BF = mybir.dt.bfloat16
AF = mybir.ActivationFunctionType
ALU = mybir.AluOpType

B = 16        # local batch per core
L = 256       # timesteps
D = 512       # input dim
H = 512       # hidden
NG = 4 * H    # gate width
TOK = L * B   # tokens per core
NM = TOK // 128

_CACHED_NC = None


def _build():
    nc = bacc.Bacc("TRN2", target_bir_lowering=False, debug=False)

    xT = nc.dram_tensor("xT", [D, TOK], BF, kind="ExternalInput").ap()
    W = nc.dram_tensor("W", [D + H, NG], BF, kind="ExternalInput").ap()
    bias = nc.dram_tensor("bias", [1, NG], BF, kind="ExternalInput").ap()
    out_h = nc.dram_tensor("out_h", [L, B, H], BF, kind="ExternalOutput").ap()

    with tile.TileContext(nc, trace_sim=False) as tc:
        with tc.tile_pool(name="wpool", bufs=1) as wpool, \
             tc.tile_pool(name="cpool", bufs=1) as cpool:
            W_t = []
            for k in range(8):
                wt = wpool.tile([128, NG], BF, tag=f"w{k}", name=f"w{k}")
                nc.sync.dma_start(wt[:], W[128 * k:128 * (k + 1), :])
                W_t.append(wt)
            bias_t = wpool.tile([1, NG], BF)
            nc.sync.dma_start(bias_t[:], bias[:, :])
            ones_t = cpool.tile([1, 128], BF)
            nc.vector.memset(ones_t[:, :], 1.0)
            ident = cpool.tile([B, B], BF)
            make_identity(nc, ident[:, :])
            # zero stationary for HAM-warmth filler matmuls (adds 0 to psum)
            zero_t = cpool.tile([1, B], BF)
            nc.vector.memset(zero_t[:, :], 0.0)

            with tc.tile_pool(name="p1x", bufs=3) as p1x, \
                 tc.tile_pool(name="xsp", bufs=5) as xsp, \
                 tc.tile_pool(name="p1ps", bufs=2, space="PSUM") as p1ps, \
                 tc.tile_pool(name="xpp", bufs=3) as xpp, \
                 tc.tile_pool(name="st", bufs=2) as st, \
                 tc.tile_pool(name="ch", bufs=2) as ch, \
                 tc.tile_pool(name="gps", bufs=2, space="PSUM") as gps, \
                 tc.tile_pool(name="tps", bufs=2, space="PSUM") as tps:

                xps = {}
                xm_map = {}
                p1n = [0]

                def emit_p1_part(m, n):
                    if n == 0:
                        xps[m] = xsp.tile([128, NG], BF, tag="xps",
                                          name=f"xps{m}")
                        xm = p1x.tile([128, 4, 128], BF, tag="xm", name="xm")
                        src = xT[:, 128 * m:128 * (m + 1)]
                        nc.gpsimd.dma_start(
                            xm[:, :, :],
                            src.rearrange("(k p) c -> p k c", k=4))
                        xm_map[m] = xm
                    xm = xm_map[m]
                    ps = p1ps.tile([128, 512], FP, tag="ps1", name="ps1")
                    for k in range(4):
                        nc.tensor.matmul(
                            ps[:, :], xm[:, k, :],
                            W_t[k][:, 512 * n:512 * (n + 1)],
                            start=(k == 0), stop=False)
                    nc.tensor.matmul(
                        ps[:, :], ones_t[:, :],
                        bias_t[:, 512 * n:512 * (n + 1)],
                        start=False, stop=True)
                    # psum->sbuf stage; alternate ScalarE/DVE
                    if p1n[0] % 2 == 0:
                        nc.scalar.copy(
                            xps[m][:, 512 * n:512 * (n + 1)], ps[:, :])
                    else:
                        nc.vector.tensor_copy(
                            xps[m][:, 512 * n:512 * (n + 1)], ps[:, :])
                    p1n[0] += 1

                def emit_p1(m):
                    for n in range(4):
                        emit_p1_part(m, n)

                xp_t = {}

                def emit_xp(t):
                    xp = xpp.tile([B, NG], BF, tag="xp", name="xp")
                    nc.gpsimd.dma_start(
                        xp[:], xps[t // 8][B * (t % 8):B * (t % 8) + B, :])
                    xp_t[t] = xp

                c_prev = st.tile([B, H], BF, tag="c", name="c0")
                nc.vector.memset(c_prev[:, :], 0.0)
                hT_prev = st.tile([128, 4, B], BF, tag="hT", name="hT0")
                nc.vector.memset(hT_prev[:, :, :], 0.0)

                for m in range(2):
                    emit_p1(m)
                emit_xp(0)
                emit_xp(1)

                for t in range(L):
                    if t % 2 == 0 and t // 8 + 2 < NM:
                        emit_p1_part(t // 8 + 2, (t % 8) // 2)

                    xp = xp_t.pop(t)
                    P = gps.tile([128, 512], FP, tag="P", name="P")
                    for j in range(4):
                        nc.tensor.matmul(
                            P[32 * j:32 * j + B, :], ident[:, :],
                            xp[:, 512 * j:512 * (j + 1)],
                            start=True, stop=False, tile_position=(0, 32 * j))
                    # HAM-warmth fillers: zero-stationary matmuls add 0 to
                    # the psum group but keep the PE activity window busy
                    # during the previous step's activation chain.
                    for f in range(3):
                        for j in range(4):
                            nc.tensor.matmul(
                                P[32 * j:32 * j + B, :], zero_t[:, :],
                                W_t[f][0:1, 512 * j:512 * (j + 1)],
                                start=False, stop=False,
                                tile_position=(0, 32 * j))
                    for k in range(4):
                        for j in range(4):
                            nc.tensor.matmul(
                                P[32 * j:32 * j + B, :],
                                hT_prev[:, k, :],
                                W_t[4 + k][:, 512 * j:512 * (j + 1)],
                                start=False, stop=(k == 3),
                                tile_position=(0, 32 * j))

                    # strips: f@0:16, i@32:48, o@64:80, g~@96:112
                    s = ch.tile([112, H], BF, tag="s", name="s")
                    nc.scalar.activation(s[:, :], P[0:112, :], AF.Sigmoid)
                    # u = 2*g~ - 1 = tanh(x_g), at rows 32:48 to align with i
                    u = ch.tile([48, H], BF, tag="u", name="u")
                    nc.vector.tensor_scalar(
                        u[32:48, :], s[96:112, :], 2.0, -1.0,
                        op0=ALU.mult, op1=ALU.add)
                    # t1 = f*c split: low half on GpSimd (runs concurrent
                    # with u/t2 on DVE), high half on DVE
                    t1 = ch.tile([B, H], BF, tag="t1", name="t1")
                    nc.gpsimd.tensor_mul(t1[:, 0:256], s[0:B, 0:256],
                                         c_prev[:, 0:256])
                    nc.vector.tensor_mul(t1[:, 256:512], s[0:B, 256:512],
                                         c_prev[:, 256:512])
                    t2 = ch.tile([B, H], BF, tag="t2", name="t2")
                    nc.vector.tensor_mul(t2[:, :], s[32:48, :], u[32:48, :])
                    c_new = st.tile([B, H], BF, tag="c", name="c")
                    nc.vector.tensor_add(c_new[:, :], t1[:, :], t2[:, :])
                    # th at rows 64:80 to align with the o strip
                    th = ch.tile([80, H], BF, tag="th", name="th")
                    nc.scalar.activation(th[64:80, :], c_new[:, :], AF.Tanh)
                    h = st.tile([B, H], BF, tag="h", name="h")
                    nc.vector.tensor_mul(h[:, :], s[64:80, :], th[64:80, :])

                    nc.gpsimd.dma_start(out_h[t, :, :], h[:, :])

                    # 4 PE transposes into one psum strip, one DVE copy out
                    tp = tps.tile([128, 4, B], BF, tag="tp", name="tp")
                    for k in range(4):
                        nc.tensor.transpose(
                            tp[:, k, :], h[:, 128 * k:128 * (k + 1)],
                            ident[:, :])
                    hT_new = st.tile([128, 4, B], BF, tag="hT", name="hT")
                    nc.vector.tensor_copy(hT_new[:, :, :], tp[:, :, :])

                    if t + 2 < L:
                        emit_xp(t + 2)

                    c_prev = c_new
                    hT_prev = hT_new
    nc.compile()
    return nc


def _host_prepare(x_full, weights, direction, bslice):
    xs = x_full[bslice]
    if direction == "bw":
        xs = xs[:, ::-1, :]
    xT = np.ascontiguousarray(xs.transpose(2, 1, 0).reshape(D, TOK))
    Wc = np.concatenate(
        [weights[f"W_{direction}_{n}"].T for n in "fiog"], axis=1).copy()
    bc = np.concatenate(
        [weights[f"b_{direction}_{n}"] for n in "fiog"])[None, :].copy()
    # tanh fold: g strip pre-activations scaled by 2 (tanh(x) = 2*sig(2x)-1)
    Wc[:, 3 * H:] *= 2.0
    bc[:, 3 * H:] *= 2.0
    return {"xT": np.ascontiguousarray(xT).astype(ml_dtypes.bfloat16),
            "W": np.ascontiguousarray(Wc).astype(ml_dtypes.bfloat16),
            "bias": np.ascontiguousarray(bc).astype(ml_dtypes.bfloat16)}


def kernel(**inputs):
    global _CACHED_NC
    inputs = {k: np.asarray(v) for k, v in inputs.items()}
    x = inputs["x"]
    Bx, Lx, _ = x.shape
    assert (Bx, Lx) == (64, L)

    if _CACHED_NC is None:
        _CACHED_NC = _build()
    nc = _CACHED_NC

    in_maps = []
    meta = []
    for ci in range(8):
        d = "fw" if ci < 4 else "bw"
        bs = (ci % 4) * B
        in_maps.append(_host_prepare(x, inputs, d, slice(bs, bs + B)))
        meta.append((d, bs))

    res = run_bass_kernel_spmd(nc, in_maps, core_ids=list(range(8)))

    hf = np.zeros((L, Bx, H), np.float32)
    hb = np.zeros((L, Bx, H), np.float32)
    for ci in range(8):
        d, bs = meta[ci]
        oh = np.asarray(res.results[ci]["out_h"]).astype(np.float32)
        if d == "fw":
            hf[:, bs:bs + B, :] = oh
        else:
            hb[:, bs:bs + B, :] = oh[::-1]

    # faithful to the reference: stack time-major, flatten, hstack, reshape
    flat = np.concatenate([hf.reshape(-1, H), hb.reshape(-1, H)], axis=1)
    return flat.reshape(Bx, Lx, 2 * H).astype(np.float32)


# revision 32
# speedup vs baseline: 1.0931x; 1.0931x over previous
"""BiLSTM (B=64, L=256, D=512, H=512) on 8 Trainium2 NeuronCores.

Strategy: 8 cores = 2 directions x 4 batch-slices of 16 (weights replicated
per direction, sequential time loop local to each core — no cross-core
communication).  Backward-direction cores receive time-reversed x, so every
core runs the identical SPMD program.

v7 (surgical over the v2 baseline):
  - bf16 matmuls, fp32 PSUM; g-gate weights pre-scaled 2x so ONE sigmoid
    covers all four gate strips (tanh(x) = 2*sig(2x)-1).
  - single full-width tanh / h-mul / out-DMA per step (baseline split them
    into halves);
  - all 4 h-transposes land in ONE psum tile, moved by ONE DVE copy;
  - xpart-slice staging and out_h DMAs issued from GpSimd, phase-1 x
    tile loads as one rearranged DMA from GpSimd (keeps SP/ACT free);
  - phase-1 GEMM interleaved with the recurrence for PE warmth.
"""

import numpy as np
import ml_dtypes

from concourse import tile, mybir, bacc
from concourse.bass_utils import run_bass_kernel_spmd
from concourse.masks import make_identity

FP = mybir.dt.float32
BF = mybir.dt.bfloat16
AF = mybir.ActivationFunctionType
ALU = mybir.AluOpType

B = 16        # local batch per core
L = 256       # timesteps
D = 512       # input dim
H = 512       # hidden
NG = 4 * H    # gate width
TOK = L * B   # tokens per core
NM = TOK // 128

_CACHED_NC = None


def _build():
    nc = bacc.Bacc("TRN2", target_bir_lowering=False, debug=False)

    xT = nc.dram_tensor("xT", [D, TOK], BF, kind="ExternalInput").ap()
    W = nc.dram_tensor("W", [D + H, NG], BF, kind="ExternalInput").ap()
    bias = nc.dram_tensor("bias", [1, NG], BF, kind="ExternalInput").ap()
    out_h = nc.dram_tensor("out_h", [L, B, H], BF, kind="ExternalOutput").ap()

    with tile.TileContext(nc, trace_sim=False) as tc:
        with tc.tile_pool(name="wpool", bufs=1) as wpool, \
             tc.tile_pool(name="cpool", bufs=1) as cpool:
            W_t = []
            for k in range(8):
                wt = wpool.tile([128, NG], BF, tag=f"w{k}", name=f"w{k}")
                nc.sync.dma_start(wt[:], W[128 * k:128 * (k + 1), :])
                W_t.append(wt)
            bias_t = wpool.tile([1, NG], BF)
            nc.sync.dma_start(bias_t[:], bias[:, :])
            ones_t = cpool.tile([1, 128], BF)
            nc.vector.memset(ones_t[:, :], 1.0)
            ident = cpool.tile([B, B], BF)
            make_identity(nc, ident[:, :])
            # zero stationary for HAM-warmth filler matmuls (adds 0 to psum)
            zero_t = cpool.tile([1, B], BF)
            nc.vector.memset(zero_t[:, :], 0.0)

            with tc.tile_pool(name="p1x", bufs=3) as p1x, \
                 tc.tile_pool(name="xsp", bufs=5) as xsp, \
                 tc.tile_pool(name="p1ps", bufs=2, space="PSUM") as p1ps, \
                 tc.tile_pool(name="xpp", bufs=3) as xpp, \
                 tc.tile_pool(name="st", bufs=2) as st, \
                 tc.tile_pool(name="ch", bufs=2) as ch, \
                 tc.tile_pool(name="gps", bufs=2, space="PSUM") as gps, \
                 tc.tile_pool(name="tps", bufs=1, space="PSUM") as tps:

                xps = {}
                xm_map = {}
                p1n = [0]

                def emit_p1_part(m, n):
                    if n == 0:
                        xps[m] = xsp.tile([128, NG], BF, tag="xps",
                                          name=f"xps{m}")
                        xm = p1x.tile([128, 4, 128], BF, tag="xm", name="xm")
                        src = xT[:, 128 * m:128 * (m + 1)]
                        nc.gpsimd.dma_start(
                            xm[:, :, :],
                            src.rearrange("(k p) c -> p k c", k=4))
                        xm_map[m] = xm
                    xm = xm_map[m]
                    ps = p1ps.tile([128, 512], FP, tag="ps1", name="ps1")
                    for k in range(4):
                        nc.tensor.matmul(
                            ps[:, :], xm[:, k, :],
                            W_t[k][:, 512 * n:512 * (n + 1)],
                            start=(k == 0), stop=False)
                    nc.tensor.matmul(
                        ps[:, :], ones_t[:, :],
                        bias_t[:, 512 * n:512 * (n + 1)],
                        start=False, stop=True)
                    # psum->sbuf stage; alternate ScalarE/DVE
                    if p1n[0] % 2 == 0:
                        nc.scalar.copy(
                            xps[m][:, 512 * n:512 * (n + 1)], ps[:, :])
                    else:
                        nc.vector.tensor_copy(
                            xps[m][:, 512 * n:512 * (n + 1)], ps[:, :])
                    p1n[0] += 1

                def emit_p1(m):
                    for n in range(4):
                        emit_p1_part(m, n)

                xp_t = {}

                def emit_xp(t):
                    xp = xpp.tile([B, NG], BF, tag="xp", name="xp")
                    nc.gpsimd.dma_start(
                        xp[:], xps[t // 8][B * (t % 8):B * (t % 8) + B, :])
                    xp_t[t] = xp

                c_prev = st.tile([B, H], BF, tag="c", name="c0")
                nc.vector.memset(c_prev[:, :], 0.0)
                hT_prev = []
                for q in range(4):
                    h0 = st.tile([128, B], BF, tag=f"hT{q}", name=f"hT{q}_0")
                    nc.vector.memset(h0[:, :], 0.0)
                    hT_prev.append(h0)

                for m in range(2):
                    emit_p1(m)
                emit_xp(0)
                emit_xp(1)

                for t in range(L):
                    if t % 2 == 0 and t // 8 + 2 < NM:
                        emit_p1_part(t // 8 + 2, (t % 8) // 2)

                    xp = xp_t.pop(t)
                    P = gps.tile([128, 512], FP, tag="P", name="P")
                    for j in range(4):
                        nc.tensor.matmul(
                            P[32 * j:32 * j + B, :], ident[:, :],
                            xp[:, 512 * j:512 * (j + 1)],
                            start=True, stop=False, tile_position=(0, 32 * j))
                    for k in range(4):
                        for j in range(4):
                            nc.tensor.matmul(
                                P[32 * j:32 * j + B, :],
                                hT_prev[k][:, :],
                                W_t[4 + k][:, 512 * j:512 * (j + 1)],
                                start=False, stop=(k == 3),
                                tile_position=(0, 32 * j))

                    # front chain in hidden-halves: strips f@0:16, i@32:48,
                    # o@64:80, g~@96:112; u = 2*g~-1 at rows 32:48
                    s = ch.tile([112, H], BF, tag="s", name="s")
                    for hh in range(2):
                        cs = slice(256 * hh, 256 * (hh + 1))
                        nc.scalar.activation(s[:, cs], P[0:112, cs],
                                             AF.Sigmoid)
                    u = ch.tile([48, H], BF, tag="u", name="u")
                    t1 = ch.tile([B, H], BF, tag="t1", name="t1")
                    t2 = ch.tile([B, H], BF, tag="t2", name="t2")
                    c_new = st.tile([B, H], BF, tag="c", name="c")
                    for hh in range(2):
                        cs = slice(256 * hh, 256 * (hh + 1))
                        nc.vector.tensor_scalar(
                            u[32:48, cs], s[96:112, cs], 2.0, -1.0,
                            op0=ALU.mult, op1=ALU.add)
                        nc.vector.tensor_mul(t1[:, cs], s[0:B, cs],
                                             c_prev[:, cs])
                        nc.vector.tensor_mul(t2[:, cs], s[32:48, cs],
                                             u[32:48, cs])
                        nc.vector.tensor_add(c_new[:, cs], t1[:, cs],
                                             t2[:, cs])

                    # back chain in hidden-quarters: tanh -> hmul ->
                    # transpose -> copy per 128-col chunk, so wave k of the
                    # next step fires as soon as chunk k's hT is copied
                    th = ch.tile([80, H], BF, tag="th", name="th")
                    h = st.tile([B, H], BF, tag="h", name="h")
                    hT_new = []
                    for q in range(4):
                        cs = slice(128 * q, 128 * (q + 1))
                        nc.scalar.activation(th[64:80, cs], c_new[:, cs],
                                             AF.Tanh)
                        nc.vector.tensor_mul(h[:, cs], s[64:80, cs],
                                             th[64:80, cs])
                        tp = tps.tile([128, B], BF, tag=f"tp{q}",
                                      name=f"tp{q}")
                        nc.tensor.transpose(tp[:, :], h[:, cs], ident[:, :])
                        hTq = st.tile([128, B], BF, tag=f"hT{q}",
                                      name=f"hT{q}")
                        if q % 2 == 0:
                            nc.vector.tensor_copy(hTq[:, :], tp[:, :])
                        else:
                            nc.scalar.copy(hTq[:, :], tp[:, :])
                        hT_new.append(hTq)

                    nc.gpsimd.dma_start(out_h[t, :, :], h[:, :])

                    if t + 2 < L:
                        emit_xp(t + 2)

                    c_prev = c_new
                    hT_prev = hT_new
    nc.compile()
    return nc


def _host_prepare(x_full, weights, direction, bslice):
    xs = x_full[bslice]
    if direction == "bw":
        xs = xs[:, ::-1, :]
    xT = np.ascontiguousarray(xs.transpose(2, 1, 0).reshape(D, TOK))
    Wc = np.concatenate(
        [weights[f"W_{direction}_{n}"].T for n in "fiog"], axis=1).copy()
    bc = np.concatenate(
        [weights[f"b_{direction}_{n}"] for n in "fiog"])[None, :].copy()
    # tanh fold: g strip pre-activations scaled by 2 (tanh(x) = 2*sig(2x)-1)
    Wc[:, 3 * H:] *= 2.0
    bc[:, 3 * H:] *= 2.0
    return {"xT": np.ascontiguousarray(xT).astype(ml_dtypes.bfloat16),
            "W": np.ascontiguousarray(Wc).astype(ml_dtypes.bfloat16),
            "bias": np.ascontiguousarray(bc).astype(ml_dtypes.bfloat16)}


def kernel(**inputs):
    global _CACHED_NC
    inputs = {k: np.asarray(v) for k, v in inputs.items()}
    x = inputs["x"]
    Bx, Lx, _ = x.shape
    assert (Bx, Lx) == (64, L)

    if _CACHED_NC is None:
        _CACHED_NC = _build()
    nc = _CACHED_NC

    in_maps = []
    meta = []
    for ci in range(8):
        d = "fw" if ci < 4 else "bw"
        bs = (ci % 4) * B
        in_maps.append(_host_prepare(x, inputs, d, slice(bs, bs + B)))
        meta.append((d, bs))

    res = run_bass_kernel_spmd(nc, in_maps, core_ids=list(range(8)))

    hf = np.zeros((L, Bx, H), np.float32)
    hb = np.zeros((L, Bx, H), np.float32)
    for ci in range(8):
        d, bs = meta[ci]
        oh = np.asarray(res.results[ci]["out_h"]).astype(np.float32)
        if d == "fw":
            hf[:, bs:bs + B, :] = oh
        else:
            hb[:, bs:bs + B, :] = oh[::-1]

    # faithful to the reference: stack time-major, flatten, hstack, reshape
    flat = np.concatenate([hf.reshape(-1, H), hb.reshape(-1, H)], axis=1)
    return flat.reshape(Bx, Lx, 2 * H).astype(np.float32)
